# revision 42
# baseline (speedup 1.0000x reference)
"""Expert-parallel Conv1dBlock (Conv1d + GroupNorm + Mish) for Trainium2.

Strategy: 8 experts -> 8 NeuronCores. The host routes each sample to its
expert's core (MoE dispatch done as the sharding step), pads every core to a
common sample count, and each core runs an identical Bass/Tile program:

  - conv1d as matmuls over (Cin x K) contraction in a single float32r pass
    (11-bit mantissa, ~5e-4 rel err -- well inside the 2e-2 gate)
  - GroupNorm stats ride free on ACT/Pool accumulators; cross-partition
    group reduction via tiny 0/1 matmuls
  - rsqrt(var+eps) via fast-inverse-sqrt bit trick + Newton steps on DVE
  - Mish spread across ACT/Pool/DVE:
      w  = Exp(scol*y + tcol)          ACT (per-partition affine APs)
      q  = Square(w, bias=1) = (w+1)^2 ACT
      a  = q + 1                       Pool
      ra = recip_approx_fast(a)        DVE (1 op, ~51 ULP)
      rr = -2*ra + 1                   Pool
      out= (scol*y + tcol) * rr        DVE affine_mul_reduce (1 op)
"""

import sys

if "/opt/trn_rl_repo" not in sys.path:
    sys.path.insert(0, "/opt/trn_rl_repo")

import numpy as np

B, C, T = 512, 256, 256
E, KS, G = 8, 5, 8
EPS = 1e-5
HALF = C // 2  # 128, channels per partition block
GRP = C // G  # 32 channels per group
TP = T + 4  # padded time axis (2 halo columns each side)

PW = 8          # pairs per wave (stats batch)
TRACE = False   # set True (module-global) to run with NTFF profiling
LAST_EXEC_NS = None
LAST_RESULTS = None

_prog_cache = {}


def _install_trace_hook():
    import types

    if "antenv.axon_hooks" not in sys.modules:
        mod = types.ModuleType("antenv.axon_hooks")
        holder = [None]
        mod.set_axon_ntff_profile_hook = lambda h: holder.__setitem__(0, h)
        mod.get_axon_ntff_profile_hook = lambda: holder[0]
        sys.modules["antenv.axon_hooks"] = mod
        import antenv

        antenv.axon_hooks = mod
        from trn_agent_boot.trn_boot import _ntff_profile_via_ctypes

        mod.set_axon_ntff_profile_hook(
            _ntff_profile_via_ctypes("/opt/axon/libaxon_pjrt.so")
        )
    from concourse import bass_utils

    bass_utils.upload_artifacts = lambda tmpdir: f"local:{tmpdir}"


def _build_program(NP):
    import concourse.bacc as bacc
    import concourse.tile as tile
    from concourse import mybir

    dt = mybir.dt
    alu = mybir.AluOpType
    act = mybir.ActivationFunctionType

    nc = bacc.Bacc(None, target_bir_lowering=False)

    xh = nc.dram_tensor("xh", [NP, 2, HALF, 2 * TP], dt.float32r, kind="ExternalInput")
    # weights laid out [co_blk, ci_blk, ci, k, co]
    wh = nc.dram_tensor("wh", [2, 2, HALF, KS, HALF], dt.float32r, kind="ExternalInput")
    bias2 = nc.dram_tensor("bias2", [HALF, 2], dt.float32, kind="ExternalInput")
    gamma2 = nc.dram_tensor("gamma2", [HALF, 2], dt.float32, kind="ExternalInput")
    beta2 = nc.dram_tensor("beta2", [HALF, 2], dt.float32, kind="ExternalInput")
    gmat = nc.dram_tensor("gmat", [2, HALF, HALF], dt.float32r, kind="ExternalInput")
    amat = nc.dram_tensor("amat", [2, HALF, HALF], dt.float32r, kind="ExternalInput")
    yo = nc.dram_tensor("yo", [NP, 2, HALF, 2, T], dt.float32, kind="ExternalOutput")

    n_waves = (NP + PW - 1) // PW
    inv_n = 1.0 / (GRP * T)

    with tile.TileContext(nc) as tc:
        import contextlib

        with contextlib.ExitStack() as ctx:
            singles = ctx.enter_context(tc.tile_pool(name="singles", bufs=1))
            xpool = ctx.enter_context(tc.tile_pool(name="xpool", bufs=PW + 2))
            cpsum = ctx.enter_context(tc.tile_pool(name="cpsum", bufs=3, space="PSUM"))
            ybpool = ctx.enter_context(tc.tile_pool(name="ybpool", bufs=2 * PW + 2))
            y2pool = ctx.enter_context(tc.tile_pool(name="y2pool", bufs=2))
            swpool = ctx.enter_context(tc.tile_pool(name="swpool", bufs=2))
            spsum = ctx.enter_context(tc.tile_pool(name="spsum", bufs=1, space="PSUM"))
            bpsum = ctx.enter_context(tc.tile_pool(name="bpsum", bufs=1, space="PSUM"))
            statp = ctx.enter_context(tc.tile_pool(name="statp", bufs=3))
            stp = ctx.enter_context(tc.tile_pool(name="stp", bufs=3))
            mpool = ctx.enter_context(tc.tile_pool(name="mpool", bufs=3))
            scrp = ctx.enter_context(tc.tile_pool(name="scrp", bufs=4))
            otpool = ctx.enter_context(tc.tile_pool(name="otpool", bufs=4))

            # ---- constants / weights resident in SBUF ----
            wsb_h = singles.tile([HALF, 2, 2, KS, HALF], dt.float32r)
            for cb in range(2):
                for cib in range(2):
                    nc.sync.dma_start(out=wsb_h[:, cb, cib, :, :], in_=wh[cb, cib])
            bias_s = singles.tile([HALF, 2], dt.float32)
            nc.sync.dma_start(out=bias_s, in_=bias2[:, :])
            gamma_s = singles.tile([HALF, 2], dt.float32)
            nc.sync.dma_start(out=gamma_s, in_=gamma2[:, :])
            beta_s = singles.tile([HALF, 2], dt.float32)
            nc.sync.dma_start(out=beta_s, in_=beta2[:, :])
            gmat_s = singles.tile([HALF, 2, HALF], dt.float32r)
            nc.sync.dma_start(out=gmat_s, in_=gmat.rearrange("c p g -> p c g"))
            amat_s = singles.tile([HALF, 2, HALF], dt.float32r)
            nc.sync.dma_start(out=amat_s, in_=amat.rearrange("c g p -> g c p"))
            magic_s = singles.tile([G, 2 * PW], dt.int32)
            nc.vector.memset(magic_s, 0x5F3759DF)

            # state carried between waves for the deferred stats+Mish pass
            prev_wave = None  # (list of (p, ybs, iw_base), swsum, swsq, nw2)

            def emit_mish_pair(p, ybt, iw0, scols, tcols):
                # mish(z) = z * tanh(softplus(z)) = z * (1 - 2/((1+e^z)^2+1))
                # with z = scol*y + tcol. GpSimd only runs dual-op
                # tensor_scalar (MULTIPLY,ADD) forms -- single-op/bypass forms
                # are ~10x slower on that engine. All scalar-free ops run on
                # the cb-merged [128, 1024] tile to amortise per-op overhead.
                zt = mpool.tile([HALF, 2, 2, T], dt.float32, name="zt", tag="zt")
                for cb in range(2):
                    for s in range(2):
                        iw = iw0 + s
                        nc.gpsimd.tensor_scalar(
                            out=zt[:, cb, s, :], in0=ybt[:, cb, s, :],
                            scalar1=scols[cb][:, iw:iw + 1],
                            scalar2=tcols[cb][:, iw:iw + 1],
                            op0=alu.mult, op1=alu.add)
                ztf = zt.rearrange("p a b t -> p (a b t)")
                w = mpool.tile([HALF, 4 * T], dt.float32, name="w", tag="w")
                nc.scalar.activation(out=w, in_=ztf, func=act.Exp)
                q = mpool.tile([HALF, 4 * T], dt.float32, name="q", tag="q")
                nc.scalar.activation(out=q, in_=w, func=act.Square, bias=1.0)
                a = mpool.tile([HALF, 4 * T], dt.float32, name="a", tag="a")
                nc.gpsimd.tensor_scalar(out=a, in0=q, scalar1=1.0,
                                        scalar2=1.0, op0=alu.mult, op1=alu.add)
                ra = mpool.tile([HALF, 4 * T], dt.float32, name="ra", tag="ra")
                nc.vector.reciprocal_approx_fast(out=ra, in_=a)
                ot = otpool.tile([HALF, 2, 2, T], dt.float32, name="ot", tag="ot")
                # ot = (-2*ra + 1) * zt in one custom-DVE pass
                nc.vector.affine_mul_reduce(
                    out=ot.rearrange("p a b t -> p (a b t)"), accum_out=None,
                    in0=ra, in1=ztf, scale=-2.0, bias=1.0)
                for cb in range(2):
                    nc.sync.dma_start(out=yo[p, cb], in_=ot[:, cb])

            def emit_stats_a(sshs, nw2):
                # ---- wave statistics part A (deferred one wave; inputs were
                # copied during that wave so the PE never stalls here) ----
                # group-sum via 0/1 matmuls; single f32r precision is plenty.
                sp = spsum.tile([HALF, 2, 2 * PW], dt.float32, name="sp",
                                tag="sp")
                spf = sp.rearrange("p a b -> p (a b)")
                for cb in range(2):
                    nc.tensor.matmul(
                        spf, gmat_s[:, cb, :],
                        sshs[cb].rearrange("p a b -> p (a b)"),
                        start=(cb == 0), stop=(cb == 1))

                # the DVE chain runs while the PE starts the next conv pair
                R = statp.tile([HALF, 2, 2 * PW], dt.float32, name="R", tag="R")
                nc.vector.memset(R, 0.0)
                negmu = R[0:G, 0, :nw2]
                nc.vector.tensor_scalar(out=negmu, in0=sp[0:G, 0, :nw2],
                                        scalar1=-inv_n, scalar2=None, op0=alu.mult)
                m2e = statp.tile([G, 2 * PW], dt.float32, name="m2e", tag="m2e")
                nc.vector.tensor_scalar(out=m2e[:, :nw2], in0=sp[0:G, 1, :nw2],
                                        scalar1=inv_n, scalar2=EPS,
                                        op0=alu.mult, op1=alu.add)
                ve = statp.tile([G, 2 * PW], dt.float32, name="ve", tag="ve")
                nc.vector.tensor_tensor(out=ve[:, :nw2], in0=negmu, in1=negmu,
                                        op=alu.mult)
                nc.vector.tensor_tensor(out=ve[:, :nw2], in0=m2e[:, :nw2],
                                        in1=ve[:, :nw2], op=alu.subtract)
                # rsqrt via bit trick + Newton (all on DVE, tiny tiles)
                yi = statp.tile([G, 2 * PW], dt.int32, name="yi", tag="yi")
                nc.vector.tensor_scalar(out=yi[:, :nw2],
                                        in0=ve[:, :nw2].bitcast(dt.int32),
                                        scalar1=1, scalar2=None,
                                        op0=alu.arith_shift_right)
                nc.vector.tensor_tensor(out=yi[:, :nw2], in0=magic_s[:, :nw2],
                                        in1=yi[:, :nw2], op=alu.subtract)
                yf = yi.bitcast(dt.float32)
                xh2 = statp.tile([G, 2 * PW], dt.float32, name="xh2", tag="xh2")
                nc.vector.tensor_scalar(out=xh2[:, :nw2], in0=ve[:, :nw2],
                                        scalar1=0.5, scalar2=None, op0=alu.mult)
                aa = statp.tile([G, 2 * PW], dt.float32, name="aa", tag="aa")
                dd = statp.tile([G, 2 * PW], dt.float32, name="dd", tag="dd")
                for it in range(3):
                    nc.vector.tensor_tensor(out=aa[:, :nw2], in0=yf[:, :nw2],
                                            in1=yf[:, :nw2], op=alu.mult)
                    nc.vector.tensor_tensor(out=aa[:, :nw2], in0=xh2[:, :nw2],
                                            in1=aa[:, :nw2], op=alu.mult)
                    nc.vector.tensor_scalar(out=dd[:, :nw2], in0=aa[:, :nw2],
                                            scalar1=-1.0, scalar2=1.5,
                                            op0=alu.mult, op1=alu.add)
                    outp = R[0:G, 1, :nw2] if it == 2 else yf[:, :nw2]
                    nc.vector.tensor_tensor(out=outp, in0=yf[:, :nw2],
                                            in1=dd[:, :nw2], op=alu.mult)

                Rf = R.rearrange("p a b -> p (a b)")
                Rh = statp.tile([HALF, 2 * 2 * PW], dt.float32r, name="Rh", tag="Rh")
                nc.vector.tensor_copy(Rh, Rf)
                return Rh

            def emit_stats_b(Rh, nw2):
                # part B: broadcast matmuls + per-channel affine params;
                # emitted after the next wave's first conv pair so the PE is
                # never blocked waiting for the DVE chain above
                scols = []
                tcols = []
                bpt = bpsum.tile([HALF, 2, 2 * 2 * PW], dt.float32, name="bp",
                                 tag="bp")
                for cb in range(2):
                    nc.tensor.matmul(bpt[:, cb, :], amat_s[:, cb, :], Rh,
                                     start=(cb == 0), stop=(cb == 1))
                for cb in range(2):
                    bp = bpt[:, cb, :].rearrange("p (a b) -> p a b", a=2)
                    scol = stp.tile([HALF, 2 * PW], dt.float32, name=f"scol{cb}",
                                    tag=f"scol{cb}")
                    nc.vector.tensor_scalar(out=scol[:, :nw2], in0=bp[:, 1, :nw2],
                                            scalar1=gamma_s[:, cb:cb + 1],
                                            scalar2=None, op0=alu.mult)
                    tcol = stp.tile([HALF, 2 * PW], dt.float32, name=f"tcol{cb}",
                                    tag=f"tcol{cb}")
                    nc.vector.tensor_tensor(out=tcol[:, :nw2], in0=bp[:, 0, :nw2],
                                            in1=scol[:, :nw2], op=alu.mult)
                    nc.vector.tensor_scalar(out=tcol[:, :nw2], in0=tcol[:, :nw2],
                                            scalar1=beta_s[:, cb:cb + 1],
                                            scalar2=None, op0=alu.add)
                    scols.append(scol)
                    tcols.append(tcol)
                return scols, tcols

            def prefetch_x(p0, p1):
                # x is (t, s)-interleaved on the host: column 2*tp+s holds
                # sample s at (padded) time tp, so each conv tap is a single
                # contiguous 512-wide moving operand covering both samples.
                tiles = []
                for p in range(p0, p1):
                    xt_h = []
                    for cib in range(2):
                        th = xpool.tile([HALF, 2 * TP], dt.float32r,
                                        name=f"xh{cib}", tag=f"xh{cib}")
                        nc.sync.dma_start(out=th, in_=xh[p, cib])
                        xt_h.append(th)
                    tiles.append(xt_h)
                return tiles

            # tapered wave sizes: full waves first, then shrink so the tail
            # after the last conv wave is a tiny mish flush
            wave_sizes = []
            rem = NP
            while rem > PW:
                wave_sizes.append(PW)
                rem -= PW
            if rem > 3:
                wave_sizes.append(rem - 3)
                rem = 3
            while rem > 0:
                wave_sizes.append(1)
                rem -= 1

            p0 = 0
            conv_done = 0
            backlog = []  # (p, ybs, iw0, scols, tcols) ready-for-mish pairs
            xt_wave = prefetch_x(0, min(wave_sizes[0], NP))
            for wi, wsz in enumerate(wave_sizes):
                p1 = p0 + wsz
                nw2 = 2 * wsz
                swsum = [swpool.tile([HALF, 2 * PW], dt.float32, name=f"sws{cb}",
                                     tag=f"sws{cb}") for cb in range(2)]
                swsq = [swpool.tile([HALF, 2 * PW], dt.float32, name=f"swq{cb}",
                                    tag=f"swq{cb}") for cb in range(2)]
                if nw2 < 2 * PW:
                    for cb in range(2):
                        nc.vector.memset(swsum[cb], 0.0)
                        nc.scalar.memzero(swsq[cb])

                # deferred stats of the previous wave: stat-matmul inputs were
                # copied during that wave, so the PE never stalls here
                if prev_wave is not None:
                    pv_pairs, pv_sshs, pv_nw2 = prev_wave
                    pv_Rh = emit_stats_a(pv_sshs, pv_nw2)
                    pv_scols = None

                wave_pairs = []
                for p in range(p0, p1):
                    iw0 = 2 * (p - p0)
                    xt_h = xt_wave[p - p0]
                    ybt = ybpool.tile([HALF, 2, 2, T], dt.float32, name="yb",
                                      tag="yb")
                    for cb in range(2):
                        cp = cpsum.tile([HALF, 2 * T], dt.float32, name=f"cp{cb}",
                                        tag=f"cp{cb}")
                        # (t,s)-interleaved: one 512-wide matmul per tap
                        # covers both samples, halos land exactly right
                        first = True
                        for cib in range(2):
                            for k in range(KS):
                                group_last = (cib == 1 and k == KS - 1)
                                nc.tensor.matmul(
                                    cp, wsb_h[:, cb, cib, k, :],
                                    xt_h[cib][:, 2 * k:2 * k + 2 * T],
                                    start=first, stop=group_last)
                                first = False
                        cpv = cp.rearrange("p (t s) -> p t s", s=2)
                        for s in range(2):
                            # bias + PSUM->SBUF eviction + sum stat on DVE
                            # (Pool/GPSIMD cannot access PSUM on TRN2);
                            # strided read de-interleaves the samples
                            nc.vector.tensor_scalar(
                                out=ybt[:, cb, s, :], in0=cpv[:, :, s],
                                scalar1=bias_s[:, cb:cb + 1], scalar2=0.0,
                                op0=alu.add, op1=alu.add,
                                accum_out=swsum[cb][:, iw0 + s:iw0 + s + 1])
                            y2 = y2pool.tile([HALF, T], dt.float32, name="y2",
                                             tag="y2")
                            # sumsq of (y+bias): Square reads PSUM directly
                            # with the bias folded into the ACT affine.
                            nc.scalar.activation(
                                out=y2, in_=cpv[:, :, s], func=act.Square,
                                bias=bias_s[:, cb:cb + 1],
                                accum_out=swsq[cb][:, iw0 + s:iw0 + s + 1])
                    wave_pairs.append((p, ybt, iw0))

                    conv_done += 1

                    # software pipelining: the previous wave's pairs join a
                    # global mish backlog once their stats are out; drain it
                    # at a rate matched to the remaining conv pairs so every
                    # engine sees a balanced mix throughout
                    if prev_wave is not None and pv_scols is None:
                        # part B after the first conv pair: the DVE chain
                        # had a full conv pair of slack to finish
                        pv_scols, pv_tcols = emit_stats_b(pv_Rh, pv_nw2)
                        for (pp, pybs, piw0) in pv_pairs:
                            backlog.append((pp, pybs, piw0, pv_scols, pv_tcols))
                    remaining = NP - conv_done
                    if backlog:
                        n_pop = (len(backlog) + max(remaining, 1) - 1) \
                            // max(remaining, 1)
                        for _ in range(min(n_pop, len(backlog))):
                            emit_mish_pair(*backlog.pop(0))

                # accum -> f32r copies emitted now (ready before the deferred
                # stat matmuls run at the start of the NEXT wave); sum and
                # sumsq share one tile so part A is a single matmul per cb
                sshs = []
                for cb in range(2):
                    ssh = statp.tile([HALF, 2, 2 * PW], dt.float32r,
                                     name=f"ssh{cb}", tag=f"ssh{cb}")
                    nc.vector.tensor_copy(ssh[:, 0, :], swsum[cb])
                    nc.vector.tensor_copy(ssh[:, 1, :], swsq[cb])
                    sshs.append(ssh)

                # prefetch the next wave's x before the flush block so its
                # DMAs are not queued behind the mish output stores
                if wi + 1 < len(wave_sizes):
                    xt_wave = prefetch_x(p1, p1 + wave_sizes[wi + 1])

                prev_wave = (wave_pairs, sshs, nw2)
                p0 = p1

            # final flush: leftover backlog plus the last wave
            for item in backlog:
                emit_mish_pair(*item)
            pv_pairs, pv_sshs, pv_nw2 = prev_wave
            pv_Rh = emit_stats_a(pv_sshs, pv_nw2)
            pv_scols, pv_tcols = emit_stats_b(pv_Rh, pv_nw2)
            for (pp, pybs, piw0) in pv_pairs:
                emit_mish_pair(pp, pybs, piw0, pv_scols, pv_tcols)

    nc.finalize()
    return nc


def kernel(x, use_expert_i, W, b, gamma, beta):
    global LAST_EXEC_NS, LAST_RESULTS
    from concourse.bass_utils import run_bass_kernel_spmd

    if TRACE:
        _install_trace_hook()

    x = np.asarray(x, dtype=np.float32)
    u = np.asarray(use_expert_i).astype(np.int64)
    W = np.asarray(W, dtype=np.float32)
    b = np.asarray(b, dtype=np.float32)
    gamma = np.asarray(gamma, dtype=np.float32)
    beta = np.asarray(beta, dtype=np.float32)

    counts = np.bincount(u, minlength=E)
    n_max = max(int(counts.max()), 2)
    NP = (n_max + 1) // 2

    key = NP
    if key not in _prog_cache:
        _prog_cache[key] = _build_program(NP)
    nc = _prog_cache[key]

    # ---- host-side dispatch (the sharding step) ----
    idx_lists = []
    in_maps = []
    # group-indicator matrices, shared across cores
    gmat = np.zeros((2, HALF, HALF), np.float32)
    amat = np.zeros((2, HALF, HALF), np.float32)
    for cb in range(2):
        for p in range(HALF):
            g = cb * (G // 2) + p // GRP
            gmat[cb, p, g] = 1.0
            amat[cb, g, p] = 1.0

    for e in range(E):
        idx = np.nonzero(u == e)[0]
        pad_to = NP * 2
        if len(idx) == 0:
            padded = np.zeros(pad_to, np.int64)
        else:
            padded = np.concatenate([idx, np.full(pad_to - len(idx), idx[0])])
        idx_lists.append((idx, padded))

        xs = x[padded]  # [2*NP, C, T]
        # (t, s)-interleaved padded layout [NP, ci_blk, 128, T+4, 2] with
        # zero halo columns: column 2*tp+s = sample s at padded time tp
        xpad = np.zeros((NP, 2, HALF, TP, 2), np.float32)
        xv = xs.reshape(NP, 2, 2, HALF, T).transpose(0, 2, 3, 4, 1)
        xpad[:, :, :, 2:2 + T, :] = xv

        # weights [co_blk, ci_blk, ci, k, co]
        we = W[e].reshape(2, HALF, 2, HALF, KS).transpose(0, 2, 3, 4, 1)
        we = np.ascontiguousarray(we)

        in_maps.append({
            "xh": xpad.reshape(NP, 2, HALF, 2 * TP),
            "wh": we,
            "bias2": np.ascontiguousarray(b[e].reshape(2, HALF).T),
            "gamma2": np.ascontiguousarray(gamma[e].reshape(2, HALF).T),
            "beta2": np.ascontiguousarray(beta[e].reshape(2, HALF).T),
            "gmat": gmat,
            "amat": amat,
        })

    res = run_bass_kernel_spmd(nc, in_maps, list(range(E)), trace=TRACE)
    LAST_EXEC_NS = res.exec_time_ns
    LAST_RESULTS = res

    out = np.empty((B, C, T), np.float32)
    for e in range(E):
        idx, padded = idx_lists[e]
        yo = res.results[e]["yo"]  # [NP, 2, 128, 2, T]
        ye = yo.transpose(0, 3, 1, 2, 4).reshape(NP * 2, C, T)
        if len(idx):
            out[idx] = ye[: len(idx)]
    return out
